# revision 10
# baseline (speedup 1.0000x reference)
"""Trainium2 Bass kernel for the stacked per-cell gate computation.

net[b,c,o] = sum_i x[b,i] Wx[c,o,i] + bx[c,o] + sum_h h[b,h] Wh[c,o,h]
cell_input = tanh(net[..., H:]);  input_gate = sigmoid(net[..., :H])

Strategy: concat x,h -> xh [B, 2048]; concat Wx,Wh per cell -> W' [2048 in,
2048 out].  Shard the C=16 cells as 2 per NeuronCore (expert parallel).  Each
core runs a [M=4096 b, K=2048, N=4096 o] matmul with a mixed-precision K
split: the first 768 k-rows run as fp8-e4m3 DoubleRow matmuls (2x PE rate),
the remaining 1280 k-rows in bf16.  The rel-err budget (2e-2) absorbs the fp8
quantization noise (~1.76e-2 measured).  Scales are unified at 8192: fp8
operands carry x*16 / W*512, the bf16-part weights carry W*8192, the bias is
preloaded into PSUM as bias*8192 (all matmuls start=False), and a single ACT
epilogue applies func(psum/8192), writing bf16 output.
"""

import os
from contextlib import ExitStack

import numpy as np
import ml_dtypes

B = 4096
IN = 1024
H = 1024
C = 16
NCORES = 8
CPC = C // NCORES          # cells per core
K = IN + H                 # contraction dim
NDR = 3                    # fp8 DoubleRow blocks (256 k-rows each)
KF8 = NDR * 256            # 768 fp8 k-rows
NKB = 10                   # bf16 k-tiles (128 k-rows each)
KFB = NKB * 128            # 1280 bf16 k-rows
OPC = CPC * 2 * H          # output columns per core
NSLAB = OPC // 512         # 512-wide output slabs per core
SLABS_PER_CELL = (2 * H) // 512
BCHUNK = 512               # batch rows resident per xh chunk
SX = 16.0                  # fp8 x scale
SW = 512.0                 # fp8 W scale
SCALE = SX * SW            # 8192

BF16 = ml_dtypes.bfloat16
E4M3 = ml_dtypes.float8_e4m3

_CACHE = {}


def _make_tc_class(tile, mybir, ScopedClock):
    """TileContext that never emits more than one sem-wait per instruction
    (this walrus build rejects multi-wait instructions in codegen)."""

    class SplitWaitTC(tile.TileContext):
        MAXW = 1

        def _split_waits(self, inst):
            si = getattr(inst, "sync_info", None)
            if si is None or len(si.on_wait) <= self.MAXW:
                return None
            waits = list(si.on_wait)
            inst.sync_info = mybir.SyncInfo(
                on_wait=waits[: self.MAXW], on_update=list(si.on_update)
            )
            nops = []
            for i in range(self.MAXW, len(waits), self.MAXW):
                nops.append(
                    mybir.InstNoOp(
                        name=self.nc.get_next_instruction_name(),
                        engine=inst.engine,
                        bass_nofuse=True,
                        sync_info=mybir.SyncInfo(
                            on_wait=waits[i : i + self.MAXW], on_update=[]
                        ),
                    )
                )
            return nops

        def _commit_and_lower(self, inst, original_block, old_bb_map, bb_to_exit_bb):
            nops = self._split_waits(inst)
            if nops:
                for nop in nops:
                    self._commit_instruction(nop)
            return super()._commit_and_lower(
                inst, original_block, old_bb_map, bb_to_exit_bb
            )

        def _drain_and_barrier(self, tick_clock, wait_clock):
            nc = self.nc
            drain_inst = nc.sync.drain()
            wait_clock.add_sem_waits(
                drain_inst.ins, ScopedClock({None: tick_clock.global_clock})
            )
            # Hoisting surplus waits onto trailing SP nops keeps semantics:
            # SP is FIFO, and the barrier below only passes once SP has
            # cleared every wait.
            si = drain_inst.ins.sync_info
            if si is not None and len(si.on_wait) > self.MAXW:
                waits = list(si.on_wait)
                drain_inst.ins.sync_info = mybir.SyncInfo(
                    on_wait=waits[: self.MAXW], on_update=list(si.on_update)
                )
                for i in range(self.MAXW, len(waits), self.MAXW):
                    nop = nc.sync.nop(nofuse=True)
                    nop.ins.sync_info = mybir.SyncInfo(
                        on_wait=waits[i : i + self.MAXW], on_update=[]
                    )
            nc.all_engine_barrier()
            assert self.sems is not None
            popped = nc._tile_sem_poison_stack.pop()
            assert popped is self._sem_poison
            nc.clear_and_free_semaphores(list(self.sems.allocated().values()))
            nc.all_engine_barrier()

    return SplitWaitTC


def _build():
    import concourse.bass as bass
    import concourse.tile as tile
    from concourse import mybir
    from concourse.vector_clock import ScopedClock

    SplitWaitTC = _make_tc_class(tile, mybir, ScopedClock)

    f32 = mybir.dt.float32
    bf16 = mybir.dt.bfloat16
    fp8 = mybir.dt.float8e4
    AF = mybir.ActivationFunctionType
    DR = mybir.MatmulPerfMode.DoubleRow

    nc = bass.Bass("TRN2", target_bir_lowering=False, debug=False)
    # Chunk-major DRAM layouts: each SBUF load is contiguous per partition.
    xh8_ap = nc.dram_tensor(
        "xh8", [B // BCHUNK, 128, NDR, 2, BCHUNK], fp8, kind="ExternalInput"
    ).ap()
    xhb_ap = nc.dram_tensor(
        "xhb", [B // BCHUNK, 128, NKB, BCHUNK], bf16, kind="ExternalInput"
    ).ap()
    # First m-tile's lhsT duplicated in its own tiny tensor so PE can start
    # after the first weight slab instead of the whole first chunk.
    x8_00_ap = nc.dram_tensor(
        "x8_00", [128, NDR, 2, 128], fp8, kind="ExternalInput"
    ).ap()
    xb_00_ap = nc.dram_tensor(
        "xb_00", [128, NKB, 128], bf16, kind="ExternalInput"
    ).ap()
    w8_ap = nc.dram_tensor(
        "w8", [NSLAB, 128, NDR, 2, 512], fp8, kind="ExternalInput"
    ).ap()
    wb_ap = nc.dram_tensor(
        "wb", [NSLAB, 128, NKB, 512], bf16, kind="ExternalInput"
    ).ap()
    bias_ap = nc.dram_tensor(
        "bias", [NSLAB, 128, 512], f32, kind="ExternalInput"
    ).ap()
    out_ap = nc.dram_tensor("out", [B, OPC], bf16, kind="ExternalOutput").ap()

    with SplitWaitTC(nc) as tc:
        with ExitStack() as ctx:
            wpool = ctx.enter_context(tc.tile_pool(name="w", bufs=1))
            xpool = ctx.enter_context(tc.tile_pool(name="xh", bufs=2))
            bpool = ctx.enter_context(tc.tile_pool(name="bias", bufs=1))
            pspool = ctx.enter_context(tc.tile_pool(name="ps", bufs=8, space="PSUM"))
            opool = ctx.enter_context(tc.tile_pool(name="o", bufs=4))

            # One SBUF tile per weight slab so the first matmuls depend only
            # on slab 0's DMA, not the whole weight load.
            w8_slabs = []
            wb_slabs = []
            for j in range(NSLAB):
                w8_slabs.append(
                    wpool.tile([128, NDR, 2, 512], fp8, tag=f"w8_{j}", name=f"w8_{j}")
                )
                wb_slabs.append(
                    wpool.tile([128, NKB, 512], bf16, tag=f"wb_{j}", name=f"wb_{j}")
                )
            x8_first = xpool.tile(
                [128, NDR, 2, BCHUNK], fp8, tag="xh8", name="xh8_c0"
            )
            xb_first = xpool.tile(
                [128, NKB, BCHUNK], bf16, tag="xhb", name="xhb_c0"
            )
            x8_00 = bpool.tile([128, NDR, 2, 128], fp8, tag="x8_00")
            xb_00 = bpool.tile([128, NKB, 128], bf16, tag="xb_00")
            bias_sb = bpool.tile([128, NSLAB, 512], f32)
            # PE p-state warmup: ~10 junk matmuls on memset tiles ramp the
            # tensor clock to max while the first real DMAs are in flight.
            warm_l = bpool.tile([128, 128], bf16, tag="warm_l")
            warm_r = bpool.tile([128, 512], bf16, tag="warm_r")
            nc.gpsimd.memset(warm_l[:], 0)
            nc.gpsimd.memset(warm_r[:], 0)
            warm_ps = pspool.tile([128, 512], f32, tag="ps", name="warm_ps")
            for _ in range(12):
                nc.tensor.matmul(
                    warm_ps[:], warm_l[:], warm_r[:], start=True, stop=True
                )
            # Issue order = bandwidth allocation order.  Everything rides the
            # sync ring (the gpsimd DMA ring starts ~6us later); one DMA per
            # queue, so the first-needed pieces all transfer in parallel.
            # bias is split per slab so the first DVE preload only waits on a
            # 256KB piece, not 2MB.
            nc.sync.dma_start(bias_sb[:, 0, :], bias_ap[0])
            nc.sync.dma_start(w8_slabs[0][:], w8_ap[0])
            nc.sync.dma_start(x8_00[:], x8_00_ap[:])
            nc.sync.dma_start(xb_00[:], xb_00_ap[:])
            nc.sync.dma_start(wb_slabs[0][:, :3, :], wb_ap[0, :, :3, :])
            nc.sync.dma_start(wb_slabs[0][:, 3:5, :], wb_ap[0, :, 3:5, :])
            nc.sync.dma_start(wb_slabs[0][:, 5:8, :], wb_ap[0, :, 5:8, :])
            nc.sync.dma_start(wb_slabs[0][:, 8:, :], wb_ap[0, :, 8:, :])
            nc.sync.dma_start(x8_first[:], xh8_ap[0])
            nc.sync.dma_start(xb_first[:], xhb_ap[0])
            nc.sync.dma_start(w8_slabs[1][:], w8_ap[1])
            nc.sync.dma_start(wb_slabs[1][:], wb_ap[1])
            nc.sync.dma_start(bias_sb[:, 1, :], bias_ap[1])
            # Remaining slabs in PROCESSING order (0,1,4,5,2,3,6,7).
            for j in (4, 5, 2, 3, 6, 7):
                nc.sync.dma_start(w8_slabs[j][:], w8_ap[j])
                nc.sync.dma_start(wb_slabs[j][:], wb_ap[j])
                nc.sync.dma_start(bias_sb[:, j, :], bias_ap[j])

            for mc in range(B // BCHUNK):
                if mc == 0:
                    x8_sb, xb_sb = x8_first, xb_first
                else:
                    x8_sb = xpool.tile(
                        [128, NDR, 2, BCHUNK], fp8, tag="xh8", name=f"xh8_c{mc}"
                    )
                    xb_sb = xpool.tile(
                        [128, NKB, BCHUNK], bf16, tag="xhb", name=f"xhb_c{mc}"
                    )
                    nc.sync.dma_start(x8_sb[:], xh8_ap[mc])
                    nc.sync.dma_start(xb_sb[:], xhb_ap[mc])
                # Slab order groups same-activation slabs (sigmoid: 0,1,4,5;
                # tanh: 2,3,6,7) so the ACT engine reloads its function table
                # 2x per chunk instead of 4x.
                for n in (0, 1, 4, 5, 2, 3, 6, 7):
                    func = (
                        AF.Sigmoid
                        if (n % SLABS_PER_CELL) < SLABS_PER_CELL // 2
                        else AF.Tanh
                    )
                    last_block = mc == B // BCHUNK - 1 and n == 7
                    for mi in range(BCHUNK // 128):
                        # Split the very last block into two column halves so
                        # half its epilogue overlaps the other half's matmuls,
                        # shrinking the exposed kernel tail.
                        nsplit = 2 if (last_block and mi == BCHUNK // 128 - 1) else 1
                        width = 512 // nsplit
                        row0 = mc * BCHUNK + mi * 128
                        for sp in range(nsplit):
                            c0 = sp * width
                            ps = pspool.tile(
                                [128, width],
                                f32,
                                tag="ps",
                                name=f"ps_{mc}_{n}_{mi}_{sp}",
                            )
                            # Preload bias*8192; every matmul accumulates.
                            nc.vector.tensor_copy(
                                ps[:],
                                bias_sb[:, n, c0 : c0 + width],
                            )
                            for j in range(NDR):
                                if mc == 0 and mi == 0:
                                    lhsT = x8_00[:, j, :, :]
                                else:
                                    lhsT = x8_sb[
                                        :, j, :, mi * 128 : (mi + 1) * 128
                                    ]
                                nc.tensor.matmul(
                                    ps[:],
                                    lhsT,
                                    w8_slabs[n][
                                        :, j, :, c0 : c0 + width
                                    ],
                                    start=False,
                                    stop=False,
                                    perf_mode=DR,
                                    skip_group_check=True,
                                )
                            for t in range(NKB):
                                if mc == 0 and mi == 0:
                                    lhsT = xb_00[:, t, :]
                                else:
                                    lhsT = xb_sb[
                                        :, t, mi * 128 : (mi + 1) * 128
                                    ]
                                nc.tensor.matmul(
                                    ps[:],
                                    lhsT,
                                    wb_slabs[n][:, t, c0 : c0 + width],
                                    start=False,
                                    stop=(t == NKB - 1),
                                    skip_group_check=True,
                                )
                            o_t = opool.tile([128, width], bf16, tag="o")
                            nc.scalar.activation(
                                o_t[:], ps[:], func, scale=1.0 / SCALE
                            )
                            nc.sync.dma_start(
                                out_ap[
                                    row0 : row0 + 128,
                                    n * 512 + c0 : n * 512 + c0 + width,
                                ],
                                o_t[:],
                            )
    return nc


def _install_ntff_hook():
    """Recreate the missing antenv.axon_hooks module so trace=True works."""
    import sys, types, ctypes, contextlib

    if "antenv.axon_hooks" in sys.modules:
        return
    so_path = "/opt/axon/libaxon_pjrt.so"
    lib = ctypes.CDLL(so_path)
    if not hasattr(lib, "axon_start_nrt_profile"):
        return
    lib.axon_start_nrt_profile.argtypes = [
        ctypes.POINTER(ctypes.c_int64),
        ctypes.c_size_t,
    ]
    lib.axon_start_nrt_profile.restype = ctypes.c_int64
    lib.axon_stop_nrt_profile.argtypes = [ctypes.c_char_p]
    lib.axon_stop_nrt_profile.restype = ctypes.c_int64

    @contextlib.contextmanager
    def _hook(output_dir, device_ids):
        import jax

        jax.devices()
        if device_ids:
            ids = (ctypes.c_int64 * len(device_ids))(*device_ids)
            rc = lib.axon_start_nrt_profile(ids, len(device_ids))
        else:
            rc = lib.axon_start_nrt_profile(None, 0)
        if rc != 0:
            raise RuntimeError(f"axon_start_nrt_profile rc={rc}")
        try:
            yield
        finally:
            n = lib.axon_stop_nrt_profile(str(output_dir).encode())
            if n < 0:
                raise RuntimeError(f"axon_stop_nrt_profile rc={n}")
            print(f"profile: {n} file(s) written to {output_dir}")

    mod = types.ModuleType("antenv.axon_hooks")
    mod.get_axon_ntff_profile_hook = lambda: _hook
    mod.set_axon_ntff_profile_hook = lambda h: None
    sys.modules["antenv.axon_hooks"] = mod


def kernel(input_word, hidden_states, Wx, bx, Wh):
    from concourse import bass_utils

    x = np.asarray(input_word, dtype=np.float32)
    h = np.asarray(hidden_states, dtype=np.float32)
    Wx = np.asarray(Wx, dtype=np.float32)
    bx = np.asarray(bx, dtype=np.float32)
    Wh = np.asarray(Wh, dtype=np.float32)

    xh_t = np.concatenate([x, h], axis=1).T                  # [K, B]
    # fp8 part: k rows [0, KF8), quantized at x*SX.
    xq = np.ascontiguousarray(xh_t[:KF8] * SX).astype(E4M3)  # [KF8, B]
    # [KF8, B] -> [chunk, p, j, i, b'] with k = j*256 + i*128 + p
    xh8_sw = np.ascontiguousarray(
        xq.reshape(NDR, 2, 128, B // BCHUNK, BCHUNK).transpose(3, 2, 0, 1, 4)
    )
    x8_00 = np.ascontiguousarray(
        xq.reshape(NDR, 2, 128, B)[:, :, :, :128].transpose(2, 0, 1, 3)
    )
    # bf16 part: k rows [KF8, K), plain bf16 (weights carry the 8192).
    xb = xh_t[KF8:].astype(BF16)                             # [KFB, B]
    xhb_sw = np.ascontiguousarray(
        xb.reshape(NKB, 128, B // BCHUNK, BCHUNK).transpose(2, 1, 0, 3)
    )
    xb_00 = np.ascontiguousarray(
        xb.reshape(NKB, 128, B)[:, :, :128].transpose(1, 0, 2)
    )

    Wcat = np.concatenate([Wx, Wh], axis=2)                  # [C, 2H, K]
    in_maps = []
    for c0 in range(NCORES):
        wc = np.concatenate(
            [Wcat[CPC * c0 + j].T for j in range(CPC)], axis=1
        )                                                    # [K, OPC]
        w8 = np.ascontiguousarray(wc[:KF8] * SW).astype(E4M3)
        w8_sw = np.ascontiguousarray(
            w8.reshape(NDR, 2, 128, NSLAB, 512).transpose(3, 2, 0, 1, 4)
        )
        wb = np.ascontiguousarray(wc[KF8:] * SCALE).astype(BF16)
        wb_sw = np.ascontiguousarray(
            wb.reshape(NKB, 128, NSLAB, 512).transpose(2, 1, 0, 3)
        )
        bias_core = np.concatenate([bx[CPC * c0 + j] for j in range(CPC)])
        bias_b = np.ascontiguousarray(
            np.broadcast_to(
                (bias_core * SCALE).astype(np.float32).reshape(NSLAB, 1, 512),
                (NSLAB, 128, 512),
            )
        )
        in_maps.append(
            {
                "xh8": xh8_sw,
                "xhb": xhb_sw,
                "x8_00": x8_00,
                "xb_00": xb_00,
                "w8": w8_sw,
                "wb": wb_sw,
                "bias": bias_b,
            }
        )

    if "nc" not in _CACHE:
        _CACHE["nc"] = _build()
    nc = _CACHE["nc"]

    trace = bool(os.environ.get("GATE_TRACE"))
    if trace:
        _install_ntff_hook()
    res = bass_utils.run_bass_kernel_spmd(
        nc, in_maps, core_ids=list(range(NCORES)), trace=trace
    )
    _CACHE["last_result"] = res

    full = np.empty((B, C, 2 * H), np.float32)
    for c0 in range(NCORES):
        o = res.results[c0]["out"].astype(np.float32).reshape(B, CPC, 2 * H)
        for j in range(CPC):
            full[:, CPC * c0 + j, :] = o[:, j, :]
    input_gate = np.ascontiguousarray(full[:, :, :H])
    cell_input = np.ascontiguousarray(full[:, :, H:])
    return (cell_input, input_gate)


# revision 12
# speedup vs baseline: 1.0002x; 1.0002x over previous
"""Trainium2 Bass kernel for the stacked per-cell gate computation.

net[b,c,o] = sum_i x[b,i] Wx[c,o,i] + bx[c,o] + sum_h h[b,h] Wh[c,o,h]
cell_input = tanh(net[..., H:]);  input_gate = sigmoid(net[..., :H])

Strategy: concat x,h -> xh [B, 2048]; concat Wx,Wh per cell -> W' [2048 in,
2048 out].  Shard the C=16 cells as 2 per NeuronCore (expert parallel).  Each
core runs a [M=4096 b, K=2048, N=4096 o] matmul with a mixed-precision K
split: the first 768 k-rows run as fp8-e4m3 DoubleRow matmuls (2x PE rate),
the remaining 1280 k-rows in bf16.  The rel-err budget (2e-2) absorbs the fp8
quantization noise (~1.76e-2 measured).  Scales are unified at 8192: fp8
operands carry x*16 / W*512, the bf16-part weights carry W*8192, the bias is
preloaded into PSUM as bias*8192 (all matmuls start=False), and a single ACT
epilogue applies func(psum/8192), writing bf16 output.
"""

import os
from contextlib import ExitStack

import numpy as np
import ml_dtypes

B = 4096
IN = 1024
H = 1024
C = 16
NCORES = 8
CPC = C // NCORES          # cells per core
K = IN + H                 # contraction dim
NDR = 3                    # fp8 DoubleRow blocks (256 k-rows each)
KF8 = NDR * 256            # 768 fp8 k-rows
NKB = 10                   # bf16 k-tiles (128 k-rows each)
KFB = NKB * 128            # 1280 bf16 k-rows
OPC = CPC * 2 * H          # output columns per core
NSLAB = OPC // 512         # 512-wide output slabs per core
SLABS_PER_CELL = (2 * H) // 512
BCHUNK = 512               # batch rows resident per xh chunk
SX = 16.0                  # fp8 x scale
SW = 512.0                 # fp8 W scale
SCALE = SX * SW            # 8192

BF16 = ml_dtypes.bfloat16
E4M3 = ml_dtypes.float8_e4m3

_CACHE = {}


def _make_tc_class(tile, mybir, ScopedClock):
    """TileContext that never emits more than one sem-wait per instruction
    (this walrus build rejects multi-wait instructions in codegen)."""

    class SplitWaitTC(tile.TileContext):
        MAXW = 1

        def _split_waits(self, inst):
            si = getattr(inst, "sync_info", None)
            if si is None or len(si.on_wait) <= self.MAXW:
                return None
            waits = list(si.on_wait)
            inst.sync_info = mybir.SyncInfo(
                on_wait=waits[: self.MAXW], on_update=list(si.on_update)
            )
            nops = []
            for i in range(self.MAXW, len(waits), self.MAXW):
                nops.append(
                    mybir.InstNoOp(
                        name=self.nc.get_next_instruction_name(),
                        engine=inst.engine,
                        bass_nofuse=True,
                        sync_info=mybir.SyncInfo(
                            on_wait=waits[i : i + self.MAXW], on_update=[]
                        ),
                    )
                )
            return nops

        def _commit_and_lower(self, inst, original_block, old_bb_map, bb_to_exit_bb):
            nops = self._split_waits(inst)
            if nops:
                for nop in nops:
                    self._commit_instruction(nop)
            return super()._commit_and_lower(
                inst, original_block, old_bb_map, bb_to_exit_bb
            )

        def _drain_and_barrier(self, tick_clock, wait_clock):
            nc = self.nc
            drain_inst = nc.sync.drain()
            wait_clock.add_sem_waits(
                drain_inst.ins, ScopedClock({None: tick_clock.global_clock})
            )
            # Hoisting surplus waits onto trailing SP nops keeps semantics:
            # SP is FIFO, and the barrier below only passes once SP has
            # cleared every wait.
            si = drain_inst.ins.sync_info
            if si is not None and len(si.on_wait) > self.MAXW:
                waits = list(si.on_wait)
                drain_inst.ins.sync_info = mybir.SyncInfo(
                    on_wait=waits[: self.MAXW], on_update=list(si.on_update)
                )
                for i in range(self.MAXW, len(waits), self.MAXW):
                    nop = nc.sync.nop(nofuse=True)
                    nop.ins.sync_info = mybir.SyncInfo(
                        on_wait=waits[i : i + self.MAXW], on_update=[]
                    )
            nc.all_engine_barrier()
            assert self.sems is not None
            popped = nc._tile_sem_poison_stack.pop()
            assert popped is self._sem_poison
            nc.clear_and_free_semaphores(list(self.sems.allocated().values()))
            nc.all_engine_barrier()

    return SplitWaitTC


def _build():
    import concourse.bass as bass
    import concourse.tile as tile
    from concourse import mybir
    from concourse.vector_clock import ScopedClock

    SplitWaitTC = _make_tc_class(tile, mybir, ScopedClock)

    f32 = mybir.dt.float32
    bf16 = mybir.dt.bfloat16
    fp8 = mybir.dt.float8e4
    AF = mybir.ActivationFunctionType
    DR = mybir.MatmulPerfMode.DoubleRow

    nc = bass.Bass("TRN2", target_bir_lowering=False, debug=False)
    # Chunk-major DRAM layouts: each SBUF load is contiguous per partition.
    xh8_ap = nc.dram_tensor(
        "xh8", [B // BCHUNK, 128, NDR, 2, BCHUNK], fp8, kind="ExternalInput"
    ).ap()
    xhb_ap = nc.dram_tensor(
        "xhb", [B // BCHUNK, 128, NKB, BCHUNK], bf16, kind="ExternalInput"
    ).ap()
    # First m-tile's lhsT duplicated in its own tiny tensor so PE can start
    # after the first weight slab instead of the whole first chunk.
    x8_00_ap = nc.dram_tensor(
        "x8_00", [128, NDR, 2, 128], fp8, kind="ExternalInput"
    ).ap()
    xb_00_ap = nc.dram_tensor(
        "xb_00", [128, NKB, 128], bf16, kind="ExternalInput"
    ).ap()
    w8_ap = nc.dram_tensor(
        "w8", [NSLAB, 128, NDR, 2, 512], fp8, kind="ExternalInput"
    ).ap()
    wb_ap = nc.dram_tensor(
        "wb", [NSLAB, 128, NKB, 512], bf16, kind="ExternalInput"
    ).ap()
    bias_ap = nc.dram_tensor(
        "bias", [NSLAB, 128, 512], f32, kind="ExternalInput"
    ).ap()
    out_ap = nc.dram_tensor("out", [B, OPC], bf16, kind="ExternalOutput").ap()

    with SplitWaitTC(nc) as tc:
        with ExitStack() as ctx:
            wpool = ctx.enter_context(tc.tile_pool(name="w", bufs=1))
            xpool = ctx.enter_context(tc.tile_pool(name="xh", bufs=2))
            bpool = ctx.enter_context(tc.tile_pool(name="bias", bufs=1))
            pspool = ctx.enter_context(tc.tile_pool(name="ps", bufs=8, space="PSUM"))
            opool = ctx.enter_context(tc.tile_pool(name="o", bufs=4))

            # One SBUF tile per weight slab so the first matmuls depend only
            # on slab 0's DMA, not the whole weight load.
            w8_slabs = []
            wb_slabs = []
            for j in range(NSLAB):
                w8_slabs.append(
                    wpool.tile([128, NDR, 2, 512], fp8, tag=f"w8_{j}", name=f"w8_{j}")
                )
                wb_slabs.append(
                    wpool.tile([128, NKB, 512], bf16, tag=f"wb_{j}", name=f"wb_{j}")
                )
            x8_first = xpool.tile(
                [128, NDR, 2, BCHUNK], fp8, tag="xh8", name="xh8_c0"
            )
            xb_first = xpool.tile(
                [128, NKB, BCHUNK], bf16, tag="xhb", name="xhb_c0"
            )
            x8_00 = bpool.tile([128, NDR, 2, 128], fp8, tag="x8_00")
            xb_00 = bpool.tile([128, NKB, 128], bf16, tag="xb_00")
            bias_sb = bpool.tile([128, NSLAB, 512], f32)
            # PE p-state warmup: ~10 junk matmuls on memset tiles ramp the
            # tensor clock to max while the first real DMAs are in flight.
            warm_l = bpool.tile([128, 128], bf16, tag="warm_l")
            warm_r = bpool.tile([128, 512], bf16, tag="warm_r")
            nc.gpsimd.memset(warm_l[:], 0)
            nc.gpsimd.memset(warm_r[:], 0)
            warm_ps = pspool.tile([128, 512], f32, tag="ps", name="warm_ps")
            for _ in range(12):
                nc.tensor.matmul(
                    warm_ps[:], warm_l[:], warm_r[:], start=True, stop=True
                )
            # Issue order = bandwidth allocation order.  Everything rides the
            # sync ring (the gpsimd DMA ring starts ~6us later); one DMA per
            # queue, so the first-needed pieces all transfer in parallel.
            # bias is split per slab so the first DVE preload only waits on a
            # 256KB piece, not 2MB.
            nc.sync.dma_start(bias_sb[:, 0, :], bias_ap[0])
            nc.sync.dma_start(w8_slabs[0][:], w8_ap[0])
            nc.sync.dma_start(x8_00[:], x8_00_ap[:])
            nc.sync.dma_start(xb_00[:], xb_00_ap[:])
            nc.sync.dma_start(wb_slabs[0][:, :3, :], wb_ap[0, :, :3, :])
            nc.sync.dma_start(wb_slabs[0][:, 3:5, :], wb_ap[0, :, 3:5, :])
            nc.sync.dma_start(wb_slabs[0][:, 5:8, :], wb_ap[0, :, 5:8, :])
            nc.sync.dma_start(wb_slabs[0][:, 8:, :], wb_ap[0, :, 8:, :])
            nc.sync.dma_start(x8_first[:], xh8_ap[0])
            nc.sync.dma_start(xb_first[:], xhb_ap[0])
            nc.sync.dma_start(w8_slabs[1][:], w8_ap[1])
            nc.sync.dma_start(wb_slabs[1][:], wb_ap[1])
            nc.gpsimd.dma_start(bias_sb[:, 1, :], bias_ap[1])
            # Remaining slabs in PROCESSING order (0,1,4,5,2,3,6,7).  Bias
            # pieces ride the Pool ring to keep SP issue pressure low.
            for j in (4, 5, 2, 3, 6, 7):
                nc.sync.dma_start(w8_slabs[j][:], w8_ap[j])
                nc.sync.dma_start(wb_slabs[j][:], wb_ap[j])
                nc.gpsimd.dma_start(bias_sb[:, j, :], bias_ap[j])

            for mc in range(B // BCHUNK):
                if mc == 0:
                    x8_sb, xb_sb = x8_first, xb_first
                else:
                    x8_sb = xpool.tile(
                        [128, NDR, 2, BCHUNK], fp8, tag="xh8", name=f"xh8_c{mc}"
                    )
                    xb_sb = xpool.tile(
                        [128, NKB, BCHUNK], bf16, tag="xhb", name=f"xhb_c{mc}"
                    )
                    nc.sync.dma_start(x8_sb[:], xh8_ap[mc])
                    nc.sync.dma_start(xb_sb[:], xhb_ap[mc])
                # Slab order groups same-activation slabs (sigmoid: 0,1,4,5;
                # tanh: 2,3,6,7) so the ACT engine reloads its function table
                # 2x per chunk instead of 4x.
                for n in (0, 1, 4, 5, 2, 3, 6, 7):
                    func = (
                        AF.Sigmoid
                        if (n % SLABS_PER_CELL) < SLABS_PER_CELL // 2
                        else AF.Tanh
                    )
                    last_block = mc == B // BCHUNK - 1 and n == 7
                    for mi in range(BCHUNK // 128):
                        # Split the very last block into two column halves so
                        # half its epilogue overlaps the other half's matmuls,
                        # shrinking the exposed kernel tail.
                        nsplit = 2 if (last_block and mi == BCHUNK // 128 - 1) else 1
                        width = 512 // nsplit
                        row0 = mc * BCHUNK + mi * 128
                        for sp in range(nsplit):
                            c0 = sp * width
                            ps = pspool.tile(
                                [128, width],
                                f32,
                                tag="ps",
                                name=f"ps_{mc}_{n}_{mi}_{sp}",
                            )
                            # Preload bias*8192; every matmul accumulates.
                            nc.vector.tensor_copy(
                                ps[:],
                                bias_sb[:, n, c0 : c0 + width],
                            )
                            for j in range(NDR):
                                if mc == 0 and mi == 0:
                                    lhsT = x8_00[:, j, :, :]
                                else:
                                    lhsT = x8_sb[
                                        :, j, :, mi * 128 : (mi + 1) * 128
                                    ]
                                nc.tensor.matmul(
                                    ps[:],
                                    lhsT,
                                    w8_slabs[n][
                                        :, j, :, c0 : c0 + width
                                    ],
                                    start=False,
                                    stop=False,
                                    perf_mode=DR,
                                    skip_group_check=True,
                                )
                            for t in range(NKB):
                                if mc == 0 and mi == 0:
                                    lhsT = xb_00[:, t, :]
                                else:
                                    lhsT = xb_sb[
                                        :, t, mi * 128 : (mi + 1) * 128
                                    ]
                                nc.tensor.matmul(
                                    ps[:],
                                    lhsT,
                                    wb_slabs[n][:, t, c0 : c0 + width],
                                    start=False,
                                    stop=(t == NKB - 1),
                                    skip_group_check=True,
                                )
                            o_t = opool.tile([128, width], bf16, tag="o")
                            nc.scalar.activation(
                                o_t[:], ps[:], func, scale=1.0 / SCALE
                            )
                            # Out DMAs issue from the (otherwise idle) Pool
                            # engine so SP's DIRECT2D issue latency never
                            # backs up the o-buffer/PSUM recycling.
                            nc.gpsimd.dma_start(
                                out_ap[
                                    row0 : row0 + 128,
                                    n * 512 + c0 : n * 512 + c0 + width,
                                ],
                                o_t[:],
                            )
    return nc


def _install_ntff_hook():
    """Recreate the missing antenv.axon_hooks module so trace=True works."""
    import sys, types, ctypes, contextlib

    if "antenv.axon_hooks" in sys.modules:
        return
    so_path = "/opt/axon/libaxon_pjrt.so"
    lib = ctypes.CDLL(so_path)
    if not hasattr(lib, "axon_start_nrt_profile"):
        return
    lib.axon_start_nrt_profile.argtypes = [
        ctypes.POINTER(ctypes.c_int64),
        ctypes.c_size_t,
    ]
    lib.axon_start_nrt_profile.restype = ctypes.c_int64
    lib.axon_stop_nrt_profile.argtypes = [ctypes.c_char_p]
    lib.axon_stop_nrt_profile.restype = ctypes.c_int64

    @contextlib.contextmanager
    def _hook(output_dir, device_ids):
        import jax

        jax.devices()
        if device_ids:
            ids = (ctypes.c_int64 * len(device_ids))(*device_ids)
            rc = lib.axon_start_nrt_profile(ids, len(device_ids))
        else:
            rc = lib.axon_start_nrt_profile(None, 0)
        if rc != 0:
            raise RuntimeError(f"axon_start_nrt_profile rc={rc}")
        try:
            yield
        finally:
            n = lib.axon_stop_nrt_profile(str(output_dir).encode())
            if n < 0:
                raise RuntimeError(f"axon_stop_nrt_profile rc={n}")
            print(f"profile: {n} file(s) written to {output_dir}")

    mod = types.ModuleType("antenv.axon_hooks")
    mod.get_axon_ntff_profile_hook = lambda: _hook
    mod.set_axon_ntff_profile_hook = lambda h: None
    sys.modules["antenv.axon_hooks"] = mod


def kernel(input_word, hidden_states, Wx, bx, Wh):
    from concourse import bass_utils

    x = np.asarray(input_word, dtype=np.float32)
    h = np.asarray(hidden_states, dtype=np.float32)
    Wx = np.asarray(Wx, dtype=np.float32)
    bx = np.asarray(bx, dtype=np.float32)
    Wh = np.asarray(Wh, dtype=np.float32)

    xh_t = np.concatenate([x, h], axis=1).T                  # [K, B]
    # fp8 part: k rows [0, KF8), quantized at x*SX.
    xq = np.ascontiguousarray(xh_t[:KF8] * SX).astype(E4M3)  # [KF8, B]
    # [KF8, B] -> [chunk, p, j, i, b'] with k = j*256 + i*128 + p
    xh8_sw = np.ascontiguousarray(
        xq.reshape(NDR, 2, 128, B // BCHUNK, BCHUNK).transpose(3, 2, 0, 1, 4)
    )
    x8_00 = np.ascontiguousarray(
        xq.reshape(NDR, 2, 128, B)[:, :, :, :128].transpose(2, 0, 1, 3)
    )
    # bf16 part: k rows [KF8, K), plain bf16 (weights carry the 8192).
    xb = xh_t[KF8:].astype(BF16)                             # [KFB, B]
    xhb_sw = np.ascontiguousarray(
        xb.reshape(NKB, 128, B // BCHUNK, BCHUNK).transpose(2, 1, 0, 3)
    )
    xb_00 = np.ascontiguousarray(
        xb.reshape(NKB, 128, B)[:, :, :128].transpose(1, 0, 2)
    )

    Wcat = np.concatenate([Wx, Wh], axis=2)                  # [C, 2H, K]
    in_maps = []
    for c0 in range(NCORES):
        wc = np.concatenate(
            [Wcat[CPC * c0 + j].T for j in range(CPC)], axis=1
        )                                                    # [K, OPC]
        w8 = np.ascontiguousarray(wc[:KF8] * SW).astype(E4M3)
        w8_sw = np.ascontiguousarray(
            w8.reshape(NDR, 2, 128, NSLAB, 512).transpose(3, 2, 0, 1, 4)
        )
        wb = np.ascontiguousarray(wc[KF8:] * SCALE).astype(BF16)
        wb_sw = np.ascontiguousarray(
            wb.reshape(NKB, 128, NSLAB, 512).transpose(2, 1, 0, 3)
        )
        bias_core = np.concatenate([bx[CPC * c0 + j] for j in range(CPC)])
        bias_b = np.ascontiguousarray(
            np.broadcast_to(
                (bias_core * SCALE).astype(np.float32).reshape(NSLAB, 1, 512),
                (NSLAB, 128, 512),
            )
        )
        in_maps.append(
            {
                "xh8": xh8_sw,
                "xhb": xhb_sw,
                "x8_00": x8_00,
                "xb_00": xb_00,
                "w8": w8_sw,
                "wb": wb_sw,
                "bias": bias_b,
            }
        )

    if "nc" not in _CACHE:
        _CACHE["nc"] = _build()
    nc = _CACHE["nc"]

    trace = bool(os.environ.get("GATE_TRACE"))
    if trace:
        _install_ntff_hook()
    res = bass_utils.run_bass_kernel_spmd(
        nc, in_maps, core_ids=list(range(NCORES)), trace=trace
    )
    _CACHE["last_result"] = res

    full = np.empty((B, C, 2 * H), np.float32)
    for c0 in range(NCORES):
        o = res.results[c0]["out"].astype(np.float32).reshape(B, CPC, 2 * H)
        for j in range(CPC):
            full[:, CPC * c0 + j, :] = o[:, j, :]
    input_gate = np.ascontiguousarray(full[:, :, :H])
    cell_input = np.ascontiguousarray(full[:, :, H:])
    return (cell_input, input_gate)


# revision 13
# speedup vs baseline: 1.0109x; 1.0106x over previous
"""Trainium2 Bass kernel for the stacked per-cell gate computation.

net[b,c,o] = sum_i x[b,i] Wx[c,o,i] + bx[c,o] + sum_h h[b,h] Wh[c,o,h]
cell_input = tanh(net[..., H:]);  input_gate = sigmoid(net[..., :H])

Strategy: concat x,h -> xh [B, 2048]; concat Wx,Wh per cell -> W' [2048 in,
2048 out].  Shard the C=16 cells as 2 per NeuronCore (expert parallel).  Each
core runs a [K=2048, N=4096 o, B=4096] matmul with output features on PSUM
partitions (lhsT = W k-tiles, moving = xh batch chunks).  Mixed-precision K
split: the first 768 k-rows run as fp8-e4m3 DoubleRow matmuls (2x PE rate),
the remaining 1280 k-rows in bf16.  The rel-err budget (2e-2) absorbs the fp8
quantization noise (~1.76e-2 measured).  Scales are unified at 8192: fp8
operands carry x*16 / W*512, the bf16-part weights carry W*8192, and the ACT
epilogue computes func(psum/8192 + bias) with bias as a per-partition vector,
writing bf16 output (transposed [o, b]; the host transposes back).
"""

import os
from contextlib import ExitStack

import numpy as np
import ml_dtypes

B = 4096
IN = 1024
H = 1024
C = 16
NCORES = 8
CPC = C // NCORES          # cells per core
K = IN + H                 # contraction dim
NDR = 3                    # fp8 DoubleRow blocks (256 k-rows each)
KF8 = NDR * 256            # 768 fp8 k-rows
NKB = 10                   # bf16 k-tiles (128 k-rows each)
KFB = NKB * 128            # 1280 bf16 k-rows
OPC = CPC * 2 * H          # output rows per core (4096)
NOT = OPC // 128           # o-tiles per core (32)
BCHUNK = 512               # batch cols per psum tile
SX = 16.0                  # fp8 x scale
SW = 512.0                 # fp8 W scale
SCALE = SX * SW            # 8192

# o-tile processing order: sigmoid tiles first (o in [0,1024) of each cell:
# tiles 0-7 and 16-23), then tanh tiles — 2 ACT table loads per b-chunk.
OT_ORDER = list(range(0, 8)) + list(range(16, 24)) + list(range(8, 16)) + list(
    range(24, 32)
)

BF16 = ml_dtypes.bfloat16
E4M3 = ml_dtypes.float8_e4m3

_CACHE = {}


def _make_tc_class(tile, mybir, ScopedClock):
    """TileContext that never emits more than one sem-wait per instruction
    (this walrus build rejects multi-wait instructions in codegen)."""

    class SplitWaitTC(tile.TileContext):
        MAXW = 1

        def _split_waits(self, inst):
            si = getattr(inst, "sync_info", None)
            if si is None or len(si.on_wait) <= self.MAXW:
                return None
            waits = list(si.on_wait)
            inst.sync_info = mybir.SyncInfo(
                on_wait=waits[: self.MAXW], on_update=list(si.on_update)
            )
            nops = []
            for i in range(self.MAXW, len(waits), self.MAXW):
                nops.append(
                    mybir.InstNoOp(
                        name=self.nc.get_next_instruction_name(),
                        engine=inst.engine,
                        bass_nofuse=True,
                        sync_info=mybir.SyncInfo(
                            on_wait=waits[i : i + self.MAXW], on_update=[]
                        ),
                    )
                )
            return nops

        def _commit_and_lower(self, inst, original_block, old_bb_map, bb_to_exit_bb):
            nops = self._split_waits(inst)
            if nops:
                for nop in nops:
                    self._commit_instruction(nop)
            return super()._commit_and_lower(
                inst, original_block, old_bb_map, bb_to_exit_bb
            )

        def _drain_and_barrier(self, tick_clock, wait_clock):
            nc = self.nc
            drain_inst = nc.sync.drain()
            wait_clock.add_sem_waits(
                drain_inst.ins, ScopedClock({None: tick_clock.global_clock})
            )
            # Hoisting surplus waits onto trailing SP nops keeps semantics:
            # SP is FIFO, and the barrier below only passes once SP has
            # cleared every wait.
            si = drain_inst.ins.sync_info
            if si is not None and len(si.on_wait) > self.MAXW:
                waits = list(si.on_wait)
                drain_inst.ins.sync_info = mybir.SyncInfo(
                    on_wait=waits[: self.MAXW], on_update=list(si.on_update)
                )
                for i in range(self.MAXW, len(waits), self.MAXW):
                    nop = nc.sync.nop(nofuse=True)
                    nop.ins.sync_info = mybir.SyncInfo(
                        on_wait=waits[i : i + self.MAXW], on_update=[]
                    )
            nc.all_engine_barrier()
            assert self.sems is not None
            popped = nc._tile_sem_poison_stack.pop()
            assert popped is self._sem_poison
            nc.clear_and_free_semaphores(list(self.sems.allocated().values()))
            nc.all_engine_barrier()

    return SplitWaitTC


def _build():
    import concourse.bass as bass
    import concourse.tile as tile
    from concourse import mybir
    from concourse.vector_clock import ScopedClock

    SplitWaitTC = _make_tc_class(tile, mybir, ScopedClock)

    f32 = mybir.dt.float32
    bf16 = mybir.dt.bfloat16
    fp8 = mybir.dt.float8e4
    AF = mybir.ActivationFunctionType
    DR = mybir.MatmulPerfMode.DoubleRow

    nc = bass.Bass("TRN2", target_bir_lowering=False, debug=False)
    # xh chunk-major layouts (moving operand): k = j*256 + i*128 + p for the
    # fp8 part, k = KF8 + t*128 + p for the bf16 part.
    xh8_ap = nc.dram_tensor(
        "xh8", [B // BCHUNK, 128, NDR, 2, BCHUNK], fp8, kind="ExternalInput"
    ).ap()
    xhb_ap = nc.dram_tensor(
        "xhb", [B // BCHUNK, 128, NKB, BCHUNK], bf16, kind="ExternalInput"
    ).ap()
    # Weights per o-tile (stationary operand).
    w8_ap = nc.dram_tensor(
        "w8", [NOT, 128, NDR, 2, 128], fp8, kind="ExternalInput"
    ).ap()
    wb_ap = nc.dram_tensor(
        "wb", [NOT, 128, NKB, 128], bf16, kind="ExternalInput"
    ).ap()
    bias_ap = nc.dram_tensor("bias", [128, NOT], f32, kind="ExternalInput").ap()
    # Output transposed: element (p, ot, b) = out feature ot*128+p, batch b.
    out_ap = nc.dram_tensor("out", [128, NOT, B], bf16, kind="ExternalOutput").ap()

    with SplitWaitTC(nc) as tc:
        with ExitStack() as ctx:
            wpool = ctx.enter_context(tc.tile_pool(name="w", bufs=1))
            xpool = ctx.enter_context(tc.tile_pool(name="xh", bufs=2))
            bpool = ctx.enter_context(tc.tile_pool(name="bias", bufs=1))
            pspool = ctx.enter_context(tc.tile_pool(name="ps", bufs=8, space="PSUM"))
            opool = ctx.enter_context(tc.tile_pool(name="o", bufs=3))

            w8_tiles = []
            wb_tiles = []
            for j in range(NOT):
                w8_tiles.append(
                    wpool.tile([128, NDR, 2, 128], fp8, tag=f"w8_{j}", name=f"w8_{j}")
                )
                wb_tiles.append(
                    wpool.tile([128, NKB, 128], bf16, tag=f"wb_{j}", name=f"wb_{j}")
                )
            x8_first = xpool.tile(
                [128, NDR, 2, BCHUNK], fp8, tag="xh8", name="xh8_c0"
            )
            xb_first = xpool.tile(
                [128, NKB, BCHUNK], bf16, tag="xhb", name="xhb_c0"
            )
            bias_sb = bpool.tile([128, NOT], f32)
            # PE p-state warmup: junk matmuls on memset tiles ramp the tensor
            # clock to max while the first real DMAs are in flight.
            warm_l = bpool.tile([128, 128], bf16, tag="warm_l")
            warm_r = bpool.tile([128, 512], bf16, tag="warm_r")
            nc.gpsimd.memset(warm_l[:], 0)
            nc.gpsimd.memset(warm_r[:], 0)
            warm_ps = pspool.tile([128, 512], f32, tag="ps", name="warm_ps")
            for _ in range(8):
                nc.tensor.matmul(
                    warm_ps[:], warm_l[:], warm_r[:], start=True, stop=True
                )
            # DMA issue order = delivery order (transfers stripe across all 16
            # queues at aggregate BW, completing in issue order).  Interleave
            # chunk-0 pieces with the first o-tiles' weights to match
            # consumption; everything else follows in processing order.
            ot0, ot1, ot2, ot3 = OT_ORDER[0], OT_ORDER[1], OT_ORDER[2], OT_ORDER[3]
            nc.sync.dma_start(bias_sb[:], bias_ap[:])
            nc.sync.dma_start(w8_tiles[ot0][:], w8_ap[ot0])
            nc.sync.dma_start(x8_first[:, 0], xh8_ap[0, :, 0])
            nc.sync.dma_start(x8_first[:, 1], xh8_ap[0, :, 1])
            nc.sync.dma_start(x8_first[:, 2], xh8_ap[0, :, 2])
            nc.sync.dma_start(wb_tiles[ot0][:], wb_ap[ot0])
            nc.sync.dma_start(xb_first[:, :2], xhb_ap[0, :, :2])
            nc.sync.dma_start(xb_first[:, 2:4], xhb_ap[0, :, 2:4])
            nc.sync.dma_start(w8_tiles[ot1][:], w8_ap[ot1])
            nc.sync.dma_start(wb_tiles[ot1][:], wb_ap[ot1])
            nc.sync.dma_start(xb_first[:, 4:7], xhb_ap[0, :, 4:7])
            nc.sync.dma_start(xb_first[:, 7:], xhb_ap[0, :, 7:])
            nc.sync.dma_start(w8_tiles[ot2][:], w8_ap[ot2])
            nc.sync.dma_start(wb_tiles[ot2][:], wb_ap[ot2])
            nc.sync.dma_start(w8_tiles[ot3][:], w8_ap[ot3])
            nc.sync.dma_start(wb_tiles[ot3][:], wb_ap[ot3])
            for j in OT_ORDER[4:]:
                nc.sync.dma_start(w8_tiles[j][:], w8_ap[j])
                nc.sync.dma_start(wb_tiles[j][:], wb_ap[j])

            NBC = B // BCHUNK
            for bc in range(NBC):
                if bc == 0:
                    x8_sb, xb_sb = x8_first, xb_first
                else:
                    x8_sb = xpool.tile(
                        [128, NDR, 2, BCHUNK], fp8, tag="xh8", name=f"xh8_c{bc}"
                    )
                    xb_sb = xpool.tile(
                        [128, NKB, BCHUNK], bf16, tag="xhb", name=f"xhb_c{bc}"
                    )
                    nc.sync.dma_start(x8_sb[:], xh8_ap[bc])
                    nc.sync.dma_start(xb_sb[:], xhb_ap[bc])
                for g in range(NOT // 4):
                    ots = OT_ORDER[4 * g : 4 * g + 4]
                    last_group = bc == NBC - 1 and g == NOT // 4 - 1
                    o_wide = opool.tile([128, 4, BCHUNK], bf16, tag="o")
                    for gi, ot in enumerate(ots):
                        func = (
                            AF.Sigmoid
                            if (ot % 16) < 8
                            else AF.Tanh
                        )
                        # Split the very last tile into two column halves so
                        # half its epilogue overlaps the other half's matmuls.
                        nsplit = 2 if (last_group and gi == 3) else 1
                        width = BCHUNK // nsplit
                        for sp in range(nsplit):
                            c0 = sp * width
                            ps = pspool.tile(
                                [128, width],
                                f32,
                                tag="ps",
                                name=f"ps_{bc}_{ot}_{sp}",
                            )
                            for j in range(NDR):
                                nc.tensor.matmul(
                                    ps[:],
                                    w8_tiles[ot][:, j],
                                    x8_sb[:, j, :, c0 : c0 + width],
                                    start=(j == 0),
                                    stop=False,
                                    perf_mode=DR,
                                )
                            for t in range(NKB):
                                nc.tensor.matmul(
                                    ps[:],
                                    wb_tiles[ot][:, t],
                                    xb_sb[:, t, c0 : c0 + width],
                                    start=False,
                                    stop=(t == NKB - 1),
                                )
                            nc.scalar.activation(
                                o_wide[:, gi, c0 : c0 + width],
                                ps[:],
                                func,
                                bias=bias_sb[:, ot : ot + 1],
                                scale=1.0 / SCALE,
                            )
                    # Batched out DMA (4 o-tiles) from the Pool engine ring so
                    # SP issue pressure never backs up the o/PSUM recycling.
                    lo, hi = min(ots), max(ots) + 1
                    nc.gpsimd.dma_start(
                        out_ap[:, lo:hi, bc * BCHUNK : (bc + 1) * BCHUNK],
                        o_wide[:],
                    )
    return nc


def _install_ntff_hook():
    """Recreate the missing antenv.axon_hooks module so trace=True works."""
    import sys, types, ctypes, contextlib

    if "antenv.axon_hooks" in sys.modules:
        return
    so_path = "/opt/axon/libaxon_pjrt.so"
    lib = ctypes.CDLL(so_path)
    if not hasattr(lib, "axon_start_nrt_profile"):
        return
    lib.axon_start_nrt_profile.argtypes = [
        ctypes.POINTER(ctypes.c_int64),
        ctypes.c_size_t,
    ]
    lib.axon_start_nrt_profile.restype = ctypes.c_int64
    lib.axon_stop_nrt_profile.argtypes = [ctypes.c_char_p]
    lib.axon_stop_nrt_profile.restype = ctypes.c_int64

    @contextlib.contextmanager
    def _hook(output_dir, device_ids):
        import jax

        jax.devices()
        if device_ids:
            ids = (ctypes.c_int64 * len(device_ids))(*device_ids)
            rc = lib.axon_start_nrt_profile(ids, len(device_ids))
        else:
            rc = lib.axon_start_nrt_profile(None, 0)
        if rc != 0:
            raise RuntimeError(f"axon_start_nrt_profile rc={rc}")
        try:
            yield
        finally:
            n = lib.axon_stop_nrt_profile(str(output_dir).encode())
            if n < 0:
                raise RuntimeError(f"axon_stop_nrt_profile rc={n}")
            print(f"profile: {n} file(s) written to {output_dir}")

    mod = types.ModuleType("antenv.axon_hooks")
    mod.get_axon_ntff_profile_hook = lambda: _hook
    mod.set_axon_ntff_profile_hook = lambda h: None
    sys.modules["antenv.axon_hooks"] = mod


def kernel(input_word, hidden_states, Wx, bx, Wh):
    from concourse import bass_utils

    x = np.asarray(input_word, dtype=np.float32)
    h = np.asarray(hidden_states, dtype=np.float32)
    Wx = np.asarray(Wx, dtype=np.float32)
    bx = np.asarray(bx, dtype=np.float32)
    Wh = np.asarray(Wh, dtype=np.float32)

    xh_t = np.concatenate([x, h], axis=1).T                  # [K, B]
    # fp8 part: k rows [0, KF8), quantized at x*SX.
    xq = np.ascontiguousarray(xh_t[:KF8] * SX).astype(E4M3)  # [KF8, B]
    xh8_sw = np.ascontiguousarray(
        xq.reshape(NDR, 2, 128, B // BCHUNK, BCHUNK).transpose(3, 2, 0, 1, 4)
    )
    # bf16 part: k rows [KF8, K), plain bf16 (weights carry the 8192).
    xb = xh_t[KF8:].astype(BF16)                             # [KFB, B]
    xhb_sw = np.ascontiguousarray(
        xb.reshape(NKB, 128, B // BCHUNK, BCHUNK).transpose(2, 1, 0, 3)
    )

    Wcat = np.concatenate([Wx, Wh], axis=2)                  # [C, 2H, K]
    in_maps = []
    for c0 in range(NCORES):
        wc = np.concatenate(
            [Wcat[CPC * c0 + j].T for j in range(CPC)], axis=1
        )                                                    # [K, OPC]
        w8 = np.ascontiguousarray(wc[:KF8] * SW).astype(E4M3)
        w8_sw = np.ascontiguousarray(
            w8.reshape(NDR, 2, 128, NOT, 128).transpose(3, 2, 0, 1, 4)
        )
        wb = np.ascontiguousarray(wc[KF8:] * SCALE).astype(BF16)
        wb_sw = np.ascontiguousarray(
            wb.reshape(NKB, 128, NOT, 128).transpose(2, 1, 0, 3)
        )
        bias_core = np.concatenate([bx[CPC * c0 + j] for j in range(CPC)])
        bias_b = np.ascontiguousarray(
            bias_core.astype(np.float32).reshape(NOT, 128).T
        )
        in_maps.append(
            {
                "xh8": xh8_sw,
                "xhb": xhb_sw,
                "w8": w8_sw,
                "wb": wb_sw,
                "bias": bias_b,
            }
        )

    if "nc" not in _CACHE:
        _CACHE["nc"] = _build()
    nc = _CACHE["nc"]

    trace = bool(os.environ.get("GATE_TRACE"))
    if trace:
        _install_ntff_hook()
    res = bass_utils.run_bass_kernel_spmd(
        nc, in_maps, core_ids=list(range(NCORES)), trace=trace
    )
    _CACHE["last_result"] = res

    full = np.empty((B, C, 2 * H), np.float32)
    for c0 in range(NCORES):
        o = res.results[c0]["out"]                           # [128, NOT, B] bf16
        o = o.astype(np.float32).transpose(1, 0, 2).reshape(OPC, B)
        for j in range(CPC):
            full[:, CPC * c0 + j, :] = o[j * 2 * H : (j + 1) * 2 * H, :].T
    input_gate = np.ascontiguousarray(full[:, :, :H])
    cell_input = np.ascontiguousarray(full[:, :, H:])
    return (cell_input, input_gate)
